# revision 17
# baseline (speedup 1.0000x reference)
"""Trainium2 Bass kernel for the stacked-Chebyshev locally-connected net.

Reference computation (B=256, k=6250, d*d=4096, O=10):
    x1 = z @ (mask*T1).T
    x2 = 2*(z @ (mask*T2).T)*x1 - T0
    x3 = 2*(z @ (mask*T3).T)*x2 - x1
    out = x3 @ C_w.T + C_b

The mask is a locally-connected conv pattern: 16x16 patch, stride 2, 25x25
positions, stacked 10x.  Rows that share the same patch-row index i have a
single contiguous, 128-aligned 1024-wide support in d — grouping by i cuts
the matmul contraction from 4096 to 1024 (4x fewer MACs than dense).

Sharding: 25 i-groups over 8 cores, perfectly balanced: every core gets 3
whole consecutive groups (full slots) plus 1/8 of group 24 (a 32-column
"mini" slot whose d-window rows [3072:4096) are the same for all cores).
A full slot is 250 k-columns split into 2 k-tiles of 128 (125 real),
contracted over 8 K-chunks against a shared 11-chunk z.T window.  The
Chebyshev recurrence is elementwise in [k, B] layout (T0 is a per-partition
scalar on the scalar engine), and the k->O projection accumulates in PSUM
per core; per-core partials are summed on the host (the "reduce" of the
k-sharding).  All inputs are host-pre-arranged so every DMA is a plain 2D
copy with large packets, split across both HWDGE queues (sync + scalar).
"""

import ml_dtypes
import numpy as np

import concourse.bass as bass
import concourse.mybir as mybir
import concourse.tile as tile
from concourse import bacc
from concourse.bass_utils import run_bass_kernel_spmd

F32 = mybir.dt.float32
F32R = mybir.dt.float32r

B = 256          # batch
O = 10           # output classes
D2 = 4096        # d*d
N_CORES = 8
FULL_SLOTS = 3   # whole groups per core
SLOT_COLS = 256  # 2 k-tiles of 128 (125 real cols each)
MINI_COLS = 32   # k-columns of the shared group-24 mini slot (<=32 real)
WIN_CH = 11      # z.T window chunks per core (full slot s uses chunks s..s+7)
G_SPLIT = 24     # the group split across all 8 cores

# matmul operand dtype for the three layer matmuls.  bfloat16 halves the
# weight-stream DMA (the kernel is DMA-bound) and runs LDWEIGHTS 2x faster
# via FWL; float32r is the full-precision fallback (measured ~120 ns/matmul
# back-to-back for both).  The k->O projection always runs in float32r.
# float16 beats bfloat16 here: same 2 bytes/elem and matmul rate, but a
# 10-bit mantissa, and every operand is O(1) so the narrow fp16 exponent
# range cannot overflow.  "f32r" is the 4-byte full-precision fallback.
MM_MODE = "f16"          # one of: "f16", "bf16", "f32r"
MM_DT = {"f16": mybir.dt.float16, "bf16": mybir.dt.bfloat16,
         "f32r": F32R}[MM_MODE]

# columns of group G_SPLIT owned by each core (6x31 + 2x32 = 250)
_MINI_N = (31, 31, 31, 31, 31, 31, 32, 32)
_MINI_OFF = tuple(int(x) for x in np.cumsum((0,) + _MINI_N[:-1]))


def _group_cols(i):
    """k-column indices of patch-row-group i (order: stack-major, then j)."""
    return np.array(
        [s * 625 + i * 25 + j for s in range(10) for j in range(25)], dtype=np.int64
    )


def _build_nc():
    nc = bacc.Bacc(
        "TRN2", target_bir_lowering=False, debug=False, num_devices=N_CORES
    )
    # z.T window, chunk-blocked: chunk c = rows [c*128, (c+1)*128)
    zw = nc.dram_tensor("zw", [WIN_CH * 128, B], MM_DT, kind="ExternalInput").ap()
    # mini-slot z.T slab (z.T rows 3072:4096), chunk-blocked
    zg = nc.dram_tensor("zg", [8 * 128, B], MM_DT, kind="ExternalInput").ap()
    # full-slot weights, SBUF layout: row = slot*128+p, col = chunk*256+n
    w_dram = [
        nc.dram_tensor(f"w{l}", [FULL_SLOTS * 128, 8 * SLOT_COLS], MM_DT,
                       kind="ExternalInput").ap()
        for l in (1, 2, 3)
    ]
    # mini-slot weights for all 3 layers: col = layer*256 + chunk*32 + n
    wm_dram = nc.dram_tensor("wm", [128, 3 * 8 * MINI_COLS], MM_DT,
                             kind="ExternalInput").ap()
    # negated T0 (additive bias on the scalar engine); col = unit index
    t0n = nc.dram_tensor("t0n", [128, 8], F32, kind="ExternalInput").ap()
    cwt = nc.dram_tensor("cwt", [128, 7 * O], F32R, kind="ExternalInput").ap()
    cwn = nc.dram_tensor("cwn", [128, 7 * O], F32R, kind="ExternalInput").ap()
    out = nc.dram_tensor("out", [O, B], F32, kind="ExternalOutput").ap()

    N_UNITS = 2 * FULL_SLOTS + 1  # 6 full k-tiles + 1 mini k-tile
    N_PROJ = N_UNITS              # one projection matmul per unit

    with tile.TileContext(nc) as tc:
        with (
            tc.tile_pool(name="zpool", bufs=1) as zpool,
            tc.tile_pool(name="cpool", bufs=1) as cpool,
            tc.tile_pool(name="wpool", bufs=9) as wpool,
            tc.tile_pool(name="xpool", bufs=3) as xpool,
            tc.tile_pool(name="ppool", bufs=7, space="PSUM") as ppool,
            tc.tile_pool(name="opool", bufs=1, space="PSUM") as opool,
        ):
            engs = (nc.sync, nc.scalar)
            n_dma = 0

            def dma(dst, src):
                nonlocal n_dma
                engs[n_dma % 2].dma_start(dst, src)
                n_dma += 1

            def wload(li, s, pieces=1):
                w = wpool.tile([128, 8 * SLOT_COLS], MM_DT, tag="w")
                if pieces == 1:
                    dma(w[:], w_dram[li][s * 128:(s + 1) * 128, :])
                    return w
                q = (8 * SLOT_COLS) // pieces
                for k in range(pieces):
                    engs[k % 2].dma_start(
                        w[:, k * q:(k + 1) * q],
                        w_dram[li][s * 128:(s + 1) * 128, k * q:(k + 1) * q])
                return w

            # slot-0 layer-1 weights + the z chunks it needs come first so PE
            # starts as early as possible; everything else streams behind.
            wt = {}
            wt[(0, 0)] = wload(0, 0, pieces=4)
            zpieces = []   # (first_chunk, tile) pieces of the z window
            for c0, c1 in ((0, 2), (2, 4), (4, 8), (8, WIN_CH)):
                t = zpool.tile([128, (c1 - c0) * B], MM_DT, tag=f"z{c0}")
                dma(t[:].rearrange("p (c n) -> p c n", n=B),
                    zw[c0 * 128:c1 * 128, :].rearrange("(c p) n -> p c n", p=128))
                zpieces.append((c0, t))

            def zchunk(c):
                for c0, t in zpieces:
                    if c0 <= c < c0 + t.shape[1] // B:
                        return t[:, (c - c0) * B:(c - c0 + 1) * B]
                raise IndexError(c)

            ztiles = [zchunk(c) for c in range(WIN_CH)]

            wt[(1, 0)] = wload(1, 0)
            wt[(2, 0)] = wload(2, 0)

            t0_sb = cpool.tile([128, 8], F32, tag="t0")
            cw_sb = cpool.tile([128, 7 * O], F32R, tag="cw")
            cwn_sb = cpool.tile([128, 7 * O], F32R, tag="cwn")
            dma(t0_sb[:], t0n[:])
            dma(cw_sb[:], cwt[:])
            dma(cwn_sb[:], cwn[:])

            for li in range(3):
                wt[(li, 1)] = wload(li, 1)

            psum_o = opool.tile([O, B], F32)
            n_proj = 0

            pending = []   # deferred projection matmuls (src, col, rows, neg)

            def project(src_t, col, rows, neg):
                pending.append((src_t, col, rows, neg))

            def flush_proj():
                # emitted a layer-block late so the PE never waits on the
                # ACT/DVE producers of the projection operands
                nonlocal n_proj
                for src_t, col, rows, neg in pending:
                    n_proj += 1
                    csb = cwn_sb if neg else cw_sb
                    nc.tensor.matmul(psum_o[:],
                                     csb[0:rows, col * O:(col + 1) * O],
                                     src_t[:],
                                     start=(n_proj == 1),
                                     stop=(n_proj == N_PROJ))
                pending.clear()

            def recurrence(li, p, xs, col, rows):
                """Per-layer epilogue for one k-tile unit (rows partitions)."""
                if li == 0:
                    x1 = xpool.tile([rows, B], F32, tag="x1")
                    nc.vector.tensor_copy(x1[:], p[:])
                    xs["x1"] = x1
                elif li == 1:
                    m2 = xpool.tile([rows, B], F32, tag="m2")
                    x2 = xpool.tile([rows, B], F32, tag="x2")
                    nc.vector.tensor_mul(m2[:], p[:], xs["x1"][:])
                    nc.vector.tensor_scalar_add(x2[:], m2[:],
                                                t0_sb[0:rows, col:col + 1])
                    xs["x2"] = x2
                else:
                    m3 = xpool.tile([rows, B], F32, tag="m3")
                    x3 = xpool.tile([rows, B], F32R, tag="x3")
                    nc.vector.tensor_mul(m3[:], p[:], xs["x2"][:])
                    nc.vector.tensor_sub(x3[:], m3[:], xs["x1"][:])
                    project(x3, col, rows, neg=False)

            # mini-slot data loads behind all full-slot weights
            wm = wpool.tile([128, 3 * 8 * MINI_COLS], MM_DT, tag="wm")
            dma(wm[:], wm_dram[:])
            zgt = []
            for c0, c1 in ((0, 4), (4, 8)):
                t = zpool.tile([128, (c1 - c0) * B], MM_DT, tag=f"g{c0}")
                dma(t[:].rearrange("p (c n) -> p c n", n=B),
                    zg[c0 * 128:c1 * 128, :].rearrange("(c p) n -> p c n", p=128))
                zgt.append((c0, t))

            def zgchunk(c):
                for c0, t in zgt:
                    if c0 <= c < c0 + t.shape[1] // B:
                        return t[:, (c - c0) * B:(c - c0 + 1) * B]
                raise IndexError(c)

            def full_slot(s):
                units = [{}, {}]
                for li in range(3):
                    w = wt[(li, s)]
                    pa = ppool.tile([128, B], F32, tag="ps")
                    pb = ppool.tile([128, B], F32, tag="ps")
                    flush_proj()
                    for kc in range(8):
                        zc = ztiles[s + kc]
                        w0 = w[:, kc * SLOT_COLS:kc * SLOT_COLS + 128]
                        w1 = w[:, kc * SLOT_COLS + 128:(kc + 1) * SLOT_COLS]
                        nc.tensor.matmul(pa[:], w0, zc,
                                         start=(kc == 0), stop=(kc == 7))
                        nc.tensor.matmul(pb[:], w1, zc,
                                         start=(kc == 0), stop=(kc == 7))
                    recurrence(li, pa, units[0], 2 * s + 0, 128)
                    recurrence(li, pb, units[1], 2 * s + 1, 128)

            for li in range(3):
                wt[(li, 2)] = wload(li, 2)

            full_slot(0)
            full_slot(1)

            # mini slot between s1 and s2: its 24 matmuls keep PE busy while
            # slot-2 weights stream in
            mini = {}
            for li in range(3):
                p = ppool.tile([MINI_COLS, B], F32, tag="ps")
                flush_proj()
                for kc in range(8):
                    lhsT = wm[:, li * 8 * MINI_COLS + kc * MINI_COLS:
                              li * 8 * MINI_COLS + (kc + 1) * MINI_COLS]
                    nc.tensor.matmul(p[:], lhsT, zgchunk(kc),
                                     start=(kc == 0), stop=(kc == 7))
                recurrence(li, p, mini, 6, MINI_COLS)

            full_slot(2)
            flush_proj()

            out_sb = cpool.tile([O, B], F32, tag="out")
            nc.vector.tensor_copy(out_sb[:], psum_o[:])
            nc.scalar.dma_start(out[:], out_sb[:])

    nc.compile()
    return nc


_NC = None


def _get_nc():
    global _NC
    if _NC is None:
        _NC = _build_nc()
    return _NC


def _prepare_in_maps(z, T1, T2, T3, T0, C_w, mask):
    z = np.ascontiguousarray(np.asarray(z, dtype=np.float32).reshape(B, D2))
    T1 = np.asarray(T1, dtype=np.float32)
    T2 = np.asarray(T2, dtype=np.float32)
    T3 = np.asarray(T3, dtype=np.float32)
    T0 = np.asarray(T0, dtype=np.float32)
    C_w = np.asarray(C_w, dtype=np.float32)
    mask = np.asarray(mask, dtype=np.float32)

    np_mm = {"f16": np.float16, "bf16": ml_dtypes.bfloat16,
             "f32r": np.float32}[MM_MODE]
    zT = np.ascontiguousarray(z.T)                   # [4096, 256]
    zg = np.ascontiguousarray(zT[G_SPLIT * 128: G_SPLIT * 128 + 1024])
    Ts = (T1, T2, T3)
    scales = (1.0, 2.0, 2.0)
    g24_cols = _group_cols(G_SPLIT)
    g24_win = np.arange(128 * G_SPLIT, 128 * G_SPLIT + 1024)

    in_maps = []
    for c in range(N_CORES):
        i0 = 3 * c
        m = {
            "zw": np.ascontiguousarray(
                zT[128 * i0: 128 * i0 + WIN_CH * 128]).astype(np_mm),
            "zg": zg.astype(np_mm),
        }
        t0n = np.zeros((128, 8), np.float32)
        cwt = np.zeros((128, 7 * O), np.float32)

        # full slots: [S, 1024, 256] -> DRAM [S*128, 8*256] SBUF layout
        for l in (1, 2, 3):
            wts = np.zeros((FULL_SLOTS, 1024, SLOT_COLS), np.float32)
            T, sc = Ts[l - 1], scales[l - 1]
            for s in range(FULL_SLOTS):
                g = i0 + s
                cols = _group_cols(g)
                ix = np.ix_(cols, np.arange(128 * g, 128 * g + 1024))
                AT = (sc * T[ix] * mask[ix]).T          # [1024, 250]
                wts[s, :, 0:125] = AT[:, 0:125]
                wts[s, :, 128:253] = AT[:, 125:250]
            m[f"w{l}"] = np.ascontiguousarray(
                wts.reshape(FULL_SLOTS, 8, 128, SLOT_COLS)
                .transpose(0, 2, 1, 3)
                .reshape(FULL_SLOTS * 128, 8 * SLOT_COLS)).astype(np_mm)

        # mini slot
        nmini = _MINI_N[c]
        mcols = g24_cols[_MINI_OFF[c]:_MINI_OFF[c] + nmini]
        wm = np.zeros((128, 3, 8, MINI_COLS), np.float32)
        for li, (T, sc) in enumerate(zip(Ts, scales)):
            A = (sc * T[np.ix_(mcols, g24_win)] * mask[np.ix_(mcols, g24_win)]).T
            wm[:, li, :, 0:nmini] = A.reshape(8, 128, nmini).transpose(1, 0, 2)
        m["wm"] = np.ascontiguousarray(
            wm.reshape(128, 3 * 8 * MINI_COLS)).astype(np_mm)

        # t0 / C_w per unit: units 0..5 = full slots (2s+kt), 6 = mini
        for s in range(FULL_SLOTS):
            cols = _group_cols(i0 + s)
            t0n[0:125, 2 * s] = -T0[cols[0:125]]
            t0n[0:125, 2 * s + 1] = -T0[cols[125:250]]
            cwt[0:125, (2 * s) * O:(2 * s + 1) * O] = C_w[:, cols[0:125]].T
            cwt[0:125, (2 * s + 1) * O:(2 * s + 2) * O] = C_w[:, cols[125:250]].T
        t0n[0:nmini, 6] = -T0[mcols]
        cwt[0:nmini, 6 * O:7 * O] = C_w[:, mcols].T
        m["t0n"] = t0n
        m["cwt"] = cwt
        m["cwn"] = -cwt
        in_maps.append(m)
    return in_maps


def kernel(z, T1, T2, T3, T0, C_w, C_b, mask):
    nc = _get_nc()
    in_maps = _prepare_in_maps(z, T1, T2, T3, T0, C_w, mask)
    res = run_bass_kernel_spmd(nc, in_maps, core_ids=list(range(N_CORES)))
    total = np.zeros((O, B), np.float32)
    for c in range(N_CORES):
        total += res.results[c]["out"]
    C_b = np.asarray(C_b, dtype=np.float32)
    return (total.T + C_b).astype(np.float32)
